# revision 8
# baseline (speedup 1.0000x reference)
"""Trainium2 Bass kernel for the CNF log-prob problem (nn_CNF_55379308314905).

Math: the ConcatSquash MLP is affine in psi, so the exact Jacobian trace is
  tr(J) = sum_a g3[a] * [W3 D2 W2 D1 W1 D0 W0]_{aa},  D_i = diag((1-h_i^2)*g_i)
computed with a rank-32 chain. The first diagonal folds into host-precomputed
3-tensors G1[a][d,c] = W1[c,d]*W0[d,a], so stage-1 matmuls take the tiny
[128,B] u0 tensor as input. Remaining diagonals fuse into the (mandatory)
PSUM->SBUF evictions. Chain runs in bf16 (validated 4e-6 rel err), forward
trajectory in fp32. Data-parallel over 8 cores (512 samples each).
"""

import math
import numpy as np
import ml_dtypes

import concourse.bass as bass
import concourse.mybir as mybir
import concourse.tile as tile
from concourse import bacc
from concourse.bass_utils import run_bass_kernel_spmd

F32 = mybir.dt.float32
BF16 = mybir.dt.bfloat16
AF = mybir.ActivationFunctionType
OP = mybir.AluOpType

D = 32
WID = 128
DT = 0.05
H = -DT
T1 = 1.0
NSTEPS = 20
NT = NSTEPS + 1          # distinct t values
B = 4096
NCORES = 8
S = B // NCORES          # 512 samples per core
LOG2PI = math.log(2.0 * math.pi)

_compiled = {}


def _build_nc():
    nc = bacc.Bacc("TRN2", target_bir_lowering=False, debug=False,
                   num_devices=NCORES)

    def din(name, shape, dt=F32):
        return nc.dram_tensor(name, shape, dt, kind="ExternalInput").ap()

    xT = din("xT", [D, S])
    ubase = din("ubase", [WID, 3, S])          # gate-arg base, layers 0..2
    ubaseN = din("ubaseN", [D, S])             # layer 3 compact
    ubaseNp = din("ubaseNp", [WID, S])         # layer 3 padded (rows 32j+i)
    cbase = din("cbase", [WID, 3, S])
    cbaseN = din("cbaseN", [D, S])
    gb = din("gb", [WID, NT * 3])              # per-t gate biases, layers 0..2
    cb = din("cb", [WID, NT * 3])
    gbN = din("gbN", [D, NT])
    gbNp = din("gbNp", [WID, NT])
    cbN = din("cbN", [D, NT])
    b1s = din("b1s", [WID, 3])
    b1N = din("b1N", [D, 1])
    G1 = din("G1", [WID, 32, WID], BF16)       # [d, a, c]
    W2T = din("W2T", [WID, WID], BF16)         # [c, e] = mid_W1[1][e, c]
    w3pad = din("w3pad", [WID, 32, 32], BF16)  # [e, a, m]: col a%8 = W3[a,:]
    fw0 = din("fw0", [D, WID])                 # l0_W1^T
    fw1 = din("fw1", [WID, WID])               # mid_W1[0]^T
    fw2 = din("fw2", [WID, WID])               # mid_W1[1]^T
    fw3 = din("fw3", [WID, D])                 # lN_W1^T
    klc = din("klc", [WID, 1])                 # +0.5*|H| at live rows else 0
    prc = din("prc", [D, 1])                   # -0.5
    out_d = nc.dram_tensor("out", [1, S], F32, kind="ExternalOutput").ap()

    with tile.TileContext(nc) as tc:
        _emit(nc, tc, locals())
    nc.compile()
    return nc


def _emit(nc, tc, io):
    import contextlib
    ctx = contextlib.ExitStack()
    with ctx:
        sing = ctx.enter_context(tc.tile_pool(name="sing", bufs=1))
        gp = ctx.enter_context(tc.tile_pool(name="gp", bufs=2))
        g3p = ctx.enter_context(tc.tile_pool(name="g3p", bufs=3))
        fwp = ctx.enter_context(tc.tile_pool(name="fwp", bufs=2))
        up = ctx.enter_context(tc.tile_pool(name="up", bufs=4))
        vp = ctx.enter_context(tc.tile_pool(name="vp", bufs=3))
        psip = ctx.enter_context(tc.tile_pool(name="psip", bufs=2))
        rp = ctx.enter_context(tc.tile_pool(name="rp", bufs=2))
        ps1 = ctx.enter_context(tc.tile_pool(name="ps1", bufs=3, space="PSUM"))
        ps2 = ctx.enter_context(tc.tile_pool(name="ps2", bufs=2, space="PSUM"))
        pstau = ctx.enter_context(tc.tile_pool(name="pstau", bufs=2, space="PSUM"))
        pslog = ctx.enter_context(tc.tile_pool(name="pslog", bufs=1, space="PSUM"))

        # ---- load all inputs to SBUF ----
        def load(name, shape, dt=F32):
            t = sing.tile(shape, dt, tag=name)
            nc.sync.dma_start(out=t, in_=io[name][:])
            return t

        s_xT = load("xT", [D, S])
        s_ub = load("ubase", [WID, 3, S])
        s_ubN = load("ubaseN", [D, S])
        s_ubNp = load("ubaseNp", [WID, S])
        s_cb = load("cbase", [WID, 3, S])
        s_cbN = load("cbaseN", [D, S])
        s_gb = load("gb", [WID, NT * 3])
        s_cbias = load("cb", [WID, NT * 3])
        s_gbN = load("gbN", [D, NT])
        s_gbNp = load("gbNp", [WID, NT])
        s_cbN2 = load("cbN", [D, NT])
        s_b1s = load("b1s", [WID, 3])
        s_b1N = load("b1N", [D, 1])
        s_G1 = load("G1", [WID, 32, WID], BF16)
        s_W2T = load("W2T", [WID, WID], BF16)
        s_w3 = load("w3pad", [WID, 32, 32], BF16)
        s_fw0 = load("fw0", [D, WID])
        s_fw1 = load("fw1", [WID, WID])
        s_fw2 = load("fw2", [WID, WID])
        s_fw3 = load("fw3", [WID, D])
        s_klc = load("klc", [WID, 1])
        s_prc = load("prc", [D, 1])

        logp = pslog.tile([1, S], F32)

        # ---- gate sets ----
        def emit_gates(j):
            """gate tiles for t_j: g0..g2, c0..c2 [WID,S]; g3 compact,
            g3 padded, c3 [D,S]."""
            gs, cs = [], []
            for i in range(3):
                g = gp.tile([WID, S], F32, tag=f"g{i}")
                nc.scalar.activation(g, s_ub[:, i, :], AF.Sigmoid,
                                     bias=s_gb[:, j * 3 + i:j * 3 + i + 1])
                c = gp.tile([WID, S], F32, tag=f"c{i}")
                nc.scalar.activation(c, s_cb[:, i, :], AF.Identity,
                                     bias=s_cbias[:, j * 3 + i:j * 3 + i + 1])
                gs.append(g)
                cs.append(c)
            g3 = gp.tile([D, S], F32, tag="g3")
            nc.scalar.activation(g3, s_ubN, AF.Sigmoid,
                                 bias=s_gbN[:, j:j + 1])
            g3pad = g3p.tile([WID, S], F32, tag="g3pad")
            nc.scalar.activation(g3pad, s_ubNp, AF.Sigmoid,
                                 bias=s_gbNp[:, j:j + 1])
            c3 = gp.tile([D, S], F32, tag="c3")
            nc.scalar.activation(c3, s_cbN, AF.Identity,
                                 bias=s_cbN2[:, j:j + 1])
            return dict(gs=gs, cs=cs, g3=g3, g3pad=g3pad, c3=c3)

        # ---- forward pass of the vector field ----
        def emit_fwd(psi_in, gset):
            """returns (kp [D,S] f32, us [u0p bf16, u1p f32, u2p f32])"""
            gs, cs = gset["gs"], gset["cs"]
            us = []
            h = psi_in
            fws = [s_fw0, s_fw1, s_fw2]
            for i in range(3):
                pre = ps1.tile([WID, S], F32, tag="ch1")
                nc.tensor.matmul(pre, fws[i], h, start=True, stop=True)
                z = fwp.tile([WID, S], F32, tag=f"z{i}")
                nc.vector.scalar_tensor_tensor(
                    z, pre, s_b1s[:, i:i + 1], gs[i], OP.add, OP.mult)
                zc = fwp.tile([WID, S], F32, tag=f"zc{i}")
                nc.gpsimd.tensor_tensor(zc, z, cs[i], OP.add)
                h = fwp.tile([WID, S], F32, tag=f"h{i}")
                nc.scalar.activation(h, zc, AF.Tanh)
                hsq = fwp.tile([WID, S], F32, tag=f"hsq{i}")
                nc.scalar.activation(hsq, h, AF.Square)
                u = up.tile([WID, S], BF16 if i == 0 else F32, tag=f"u{i}")
                nc.vector.scalar_tensor_tensor(
                    u, hsq, 1.0, gs[i], OP.subtract, OP.mult)
                us.append(u)
            pre3 = ps2.tile([D, S], F32, tag="ch2")
            nc.tensor.matmul(pre3, s_fw3, h, start=True, stop=True)
            dp = fwp.tile([D, S], F32, tag="dp")
            nc.vector.scalar_tensor_tensor(
                dp, pre3, s_b1N, gset["g3"], OP.add, OP.mult)
            kp = fwp.tile([D, S], F32, tag="kp")
            nc.gpsimd.tensor_tensor(kp, dp, gset["c3"], OP.add)
            return kp, us

        # ---- trace chain for one rhs eval ----
        def emit_chain(us, gset, first):
            u0p, u1p, u2p = us
            tau = pstau.tile([WID, S], F32, tag="tau")
            # seed a -> col group (a%4)*32, lhsT col a//4; natural order
            # already cycles the 4 groups for PE concurrency
            for a in range(32):
                q1 = ps1.tile([WID, S], F32, tag="ch1")
                nc.tensor.matmul(q1, s_G1[:, a, :], u0p, start=True, stop=True)
                v1 = vp.tile([WID, S], BF16, tag="v1")
                nc.vector.tensor_tensor(v1, q1, u1p, OP.mult)
                q2 = ps2.tile([WID, S], F32, tag="ch2")
                nc.tensor.matmul(q2, s_W2T, v1, start=True, stop=True)
                if a % 2 == 0:
                    v2 = vp.tile([WID, S], BF16, tag="v2")
                    nc.vector.tensor_tensor(v2, q2, u2p, OP.mult)
                else:
                    tmp = vp.tile([WID, S], F32, tag="evt")
                    nc.scalar.copy(tmp, q2)
                    v2 = vp.tile([WID, S], BF16, tag="v2")
                    nc.gpsimd.tensor_tensor(v2, tmp, u2p, OP.mult)
                grp = (a % 4) * 32
                nc.tensor.matmul(tau[grp:grp + 32, :], s_w3[:, a, :], v2,
                                 start=(a // 4 == 0), stop=(a // 4 == 7),
                                 tile_position=(0, grp))
            r = rp.tile([WID, S], F32, tag="r")
            nc.vector.tensor_tensor(r, tau, gset["g3pad"], OP.mult)
            nc.tensor.matmul(logp, s_klc, r, start=first, stop=False)

        # ---- integration loop (software-pipelined emission) ----
        gsets = {0: emit_gates(0)}
        psiT = s_xT
        pend = []  # chains pending emission: (us, gset, first)
        for k in range(NSTEPS):
            gsets[k + 1] = emit_gates(k + 1)
            k1p, us1 = emit_fwd(psiT, gsets[k])
            pmid = psip.tile([D, S], F32, tag="pmid")
            nc.vector.scalar_tensor_tensor(
                pmid, k1p, float(H), psiT, OP.mult, OP.add)
            k2p, us2 = emit_fwd(pmid, gsets[k + 1])
            ksum = psip.tile([D, S], F32, tag="ksum")
            nc.gpsimd.tensor_tensor(ksum, k1p, k2p, OP.add)
            pnew = psip.tile([D, S], F32, tag="psiT")
            nc.vector.scalar_tensor_tensor(
                pnew, ksum, float(0.5 * H), psiT, OP.mult, OP.add)
            psiT = pnew
            pend.append((us1, gsets[k], k == 0))
            pend.append((us2, gsets[k + 1], False))
            del gsets[k]
            # emit the previous step's chains now (overlaps next forwards)
            while len(pend) > 2:
                emit_chain(*pend.pop(0))
        for args in pend:
            emit_chain(*args)

        # ---- prior + output ----
        zsq = fwp.tile([D, S], F32, tag="zsq")
        nc.scalar.activation(zsq, psiT, AF.Square)
        nc.tensor.matmul(logp, s_prc, zsq, start=False, stop=True)
        outsb = sing.tile([1, S], F32)
        nc.vector.tensor_scalar_add(outsb, logp, float(-0.5 * D * LOG2PI))
        nc.sync.dma_start(out=io["out_d"][:], in_=outsb)


def _prepare_inputs(inputs):
    """Host-side precompute. Returns per-core in_maps list."""
    f = lambda k: np.asarray(inputs[k], np.float32)
    x, cond = f("x"), f("cond")
    W0, W1m, W2m, W3 = f("l0_W1"), f("mid_W1")[0], f("mid_W1")[1], f("lN_W1")
    W2g = [f("l0_W2"), f("mid_W2")[0], f("mid_W2")[1], f("lN_W2")]
    b2g = [f("l0_b2"), f("mid_b2")[0], f("mid_b2")[1], f("lN_b2")]
    W3c = [f("l0_W3"), f("mid_W3")[0], f("mid_W3")[1], f("lN_W3")]
    b1 = [f("l0_b1"), f("mid_b1")[0], f("mid_b1")[1], f("lN_b1")]

    bf = ml_dtypes.bfloat16
    ts = (T1 + H * np.arange(NT)).astype(np.float32)        # [NT]

    # shared tensors
    shared = {}
    shared["G1"] = np.einsum("cd,da->dac", W1m, W0).astype(bf).copy()
    shared["W2T"] = W2m.T.astype(bf).copy()
    w3pad = np.zeros((WID, 32, 32), np.float32)
    for a in range(32):
        w3pad[:, a, a // 4] = W3[a, :]
    shared["w3pad"] = w3pad.astype(bf)
    shared["fw0"] = W0.T.copy()
    shared["fw1"] = W1m.T.copy()
    shared["fw2"] = W2m.T.copy()
    shared["fw3"] = W3.T.copy()
    shared["b1s"] = np.stack([b1[0], b1[1], b1[2]], axis=1).copy()
    shared["b1N"] = b1[3][:, None].copy()
    # per-t biases: gate bias = t*W2[:,0] + b2 ; c bias = t*W3[:,0]
    gbl = np.stack([np.outer(W2g[i][:, 0], ts) + b2g[i][:, None]
                    for i in range(3)], axis=2)             # [WID, NT, 3]
    shared["gb"] = gbl.reshape(WID, NT * 3).copy()
    cbl = np.stack([np.outer(W3c[i][:, 0], ts)
                    for i in range(3)], axis=2)
    shared["cb"] = cbl.reshape(WID, NT * 3).copy()
    shared["gbN"] = (np.outer(W2g[3][:, 0], ts) + b2g[3][:, None]).copy()
    shared["cbN"] = np.outer(W3c[3][:, 0], ts).copy()
    # tau row for seed a: col group (a%4)*32 + lhsT col (a//4)
    rowmap = np.array([32 * (a % 4) + a // 4 for a in range(32)])
    assert len(set(rowmap.tolist())) == 32
    gbNp = np.zeros((WID, NT), np.float32)
    klc = np.zeros((WID, 1), np.float32)
    ubNp = np.zeros((WID, B), np.float32)
    for a in range(32):
        gbNp[rowmap[a]] = shared["gbN"][a]
        klc[rowmap[a], 0] = 0.5 * DT   # +0.025  (absorbs chain sign flip)
    shared["gbNp"] = gbNp
    shared["klc"] = klc
    shared["prc"] = np.full((D, 1), -0.5, np.float32)

    # per-core tensors
    condT = cond.T                                          # [8, B]
    ub_all = np.stack([W2g[i][:, 1:] @ condT for i in range(3)], axis=1)
    cb_all = np.stack([W3c[i][:, 1:] @ condT for i in range(3)], axis=1)
    ubN_all = W2g[3][:, 1:] @ condT                         # [D, B]
    cbN_all = W3c[3][:, 1:] @ condT
    for a in range(32):
        ubNp[rowmap[a]] = ubN_all[a]

    in_maps = []
    for c in range(NCORES):
        sl = slice(c * S, (c + 1) * S)
        m = dict(shared)
        m["xT"] = x[sl].T.copy()
        m["ubase"] = ub_all[:, :, sl].copy()
        m["ubaseN"] = ubN_all[:, sl].copy()
        m["ubaseNp"] = ubNp[:, sl].copy()
        m["cbase"] = cb_all[:, :, sl].copy()
        m["cbaseN"] = cbN_all[:, sl].copy()
        in_maps.append(m)
    return in_maps


def kernel(**inputs):
    if "nc" not in _compiled:
        _compiled["nc"] = _build_nc()
    nc = _compiled["nc"]
    in_maps = _prepare_inputs(inputs)
    res = run_bass_kernel_spmd(nc, in_maps, list(range(NCORES)))
    out = np.concatenate([res.results[c]["out"][0] for c in range(NCORES)])
    return out.astype(np.float32)


if __name__ == "__main__":
    import os
    if os.path.exists("/tmp/inputs_full.npz"):
        inp = dict(np.load("/tmp/inputs_full.npz"))
    else:
        import reference as ref
        inp = {k: np.asarray(v) for k, v in ref.setup_inputs().items()}
    got = kernel(**inp)
    print("kernel output", got[:4], got.shape)


# revision 23
# speedup vs baseline: 47.0289x; 47.0289x over previous
"""Trainium2 Bass kernel for the CNF log-prob problem (nn_CNF_55379308314905).

Math: the ConcatSquash MLP is affine in psi, so the exact Jacobian trace is
  tr(J) = sum_a g3[a] * [W3 D2 W2 D1 W1 D0 W0]_{aa},  D_i = diag((1-h_i^2)*g_i)
computed with a rank-32 chain. The first diagonal folds into host-precomputed
3-tensors G1[a][d,c] = W1[c,d]*W0[d,a], so stage-1 matmuls take the tiny
[128,B] u0 tensor as input. Remaining diagonals fuse into the (mandatory)
PSUM->SBUF evictions. Chain runs in bf16 (validated 4e-6 rel err), forward
trajectory in fp32. Data-parallel over 8 cores (512 samples each).
"""

import math
import numpy as np
import ml_dtypes

import concourse.bass as bass
import concourse.mybir as mybir
import concourse.tile as tile
from concourse import bacc
from concourse.bass_utils import run_bass_kernel_spmd

F32 = mybir.dt.float32
BF16 = mybir.dt.bfloat16
AF = mybir.ActivationFunctionType
OP = mybir.AluOpType

D = 32
WID = 128
DT = 0.05
H = -DT
T1 = 1.0
NSTEPS = 20
NT = NSTEPS + 1          # distinct t values
B = 4096
NCORES = 8
S = B // NCORES          # 512 samples per core
LOG2PI = math.log(2.0 * math.pi)

# eviction route mix: 'd' = DVE direct, 'q' = ACT copy + DVE bf16 2x,
# 'p' = ACT copy + Pool.  Stage 1 evicts pairs of seeds (CH=2, 16 routes);
# stage 2 evicts single seeds (32 routes).
PAT1 = "dqp"
PAT2 = "ddqqp"
CFG = dict(ps1=2, ps2=2, tau=1, vp=4, up=4, fwp=2, gp=2, g3p=3, rp=2, psip=2, ch1=2)
ROUTES1 = [PAT1[a % len(PAT1)] for a in range(16)]
ROUTES2 = [PAT2[a % len(PAT2)] for a in range(32)]

_compiled = {}


def _build_nc(reps=1):
    nc = bacc.Bacc("TRN2", target_bir_lowering=False, debug=False,
                   num_devices=NCORES)

    def din(name, shape, dt=F32):
        return nc.dram_tensor(name, shape, dt, kind="ExternalInput").ap()

    xT = din("xT", [D, S])
    ubase = din("ubase", [WID, 3, S])          # gate-arg base, layers 0..2
    ubaseN = din("ubaseN", [D, S])             # layer 3 compact
    ubaseNp = din("ubaseNp", [WID, S])         # layer 3 padded (rows 32j+i)
    cbase = din("cbase", [WID, 3, S])
    cbaseN = din("cbaseN", [D, S])
    gb = din("gb", [WID, NT * 3])              # per-t gate biases, layers 0..2
    cb = din("cb", [WID, NT * 3])
    gbN = din("gbN", [D, NT])
    gbNp = din("gbNp", [WID, NT])
    cbN = din("cbN", [D, NT])
    b1s = din("b1s", [WID, 3])
    b1N = din("b1N", [D, 1])
    G1 = din("G1", [WID, 32, WID], BF16)       # [d, a, c]
    W2T = din("W2T", [WID, WID], BF16)         # [c, e] = mid_W1[1][e, c]
    w3pad = din("w3pad", [WID, 32, 32], BF16)  # [e, a, m]: col a%8 = W3[a,:]
    fw0 = din("fw0", [D, WID])                 # l0_W1^T
    fw1 = din("fw1", [WID, WID])               # mid_W1[0]^T
    fw2 = din("fw2", [WID, WID])               # mid_W1[1]^T
    fw3 = din("fw3", [WID, D])                 # lN_W1^T
    klc = din("klc", [WID, 1])                 # +0.5*|H| at live rows else 0
    prc = din("prc", [D, 1])                   # -0.5
    out_d = nc.dram_tensor("out", [1, S], F32, kind="ExternalOutput").ap()

    with tile.TileContext(nc) as tc:
        _emit(nc, tc, locals(), reps)
    nc.compile()
    return nc


def _emit(nc, tc, io, reps=1):
    import contextlib
    ctx = contextlib.ExitStack()
    with ctx:
        sing = ctx.enter_context(tc.tile_pool(name="sing", bufs=1))
        gp = ctx.enter_context(tc.tile_pool(name="gp", bufs=CFG["gp"]))
        g3p = ctx.enter_context(tc.tile_pool(name="g3p", bufs=CFG["g3p"]))
        fwp = ctx.enter_context(tc.tile_pool(name="fwp", bufs=CFG["fwp"]))
        up = ctx.enter_context(tc.tile_pool(name="up", bufs=CFG["up"]))
        vp = ctx.enter_context(tc.tile_pool(name="vp", bufs=CFG["vp"]))
        psip = ctx.enter_context(tc.tile_pool(name="psip", bufs=CFG["psip"]))
        rp = ctx.enter_context(tc.tile_pool(name="rp", bufs=CFG["rp"]))
        ps1 = ctx.enter_context(tc.tile_pool(name="ps1", bufs=CFG["ps1"], space="PSUM"))
        ps2 = ctx.enter_context(tc.tile_pool(name="ps2", bufs=CFG["ps2"], space="PSUM"))
        pstau = ctx.enter_context(tc.tile_pool(name="pstau", bufs=CFG["tau"], space="PSUM"))
        pslog = ctx.enter_context(tc.tile_pool(name="pslog", bufs=1, space="PSUM"))

        # ---- load all inputs to SBUF ----
        def load(name, shape, dt=F32):
            t = sing.tile(shape, dt, tag=name)
            nc.sync.dma_start(out=t, in_=io[name][:])
            return t

        s_xT = load("xT", [D, S])
        s_ub = load("ubase", [WID, 3, S])
        s_ubN = load("ubaseN", [D, S])
        s_ubNp = load("ubaseNp", [WID, S])
        s_cb = load("cbase", [WID, 3, S])
        s_cbN = load("cbaseN", [D, S])
        s_gb = load("gb", [WID, NT * 3])
        s_cbias = load("cb", [WID, NT * 3])
        s_gbN = load("gbN", [D, NT])
        s_gbNp = load("gbNp", [WID, NT])
        s_cbN2 = load("cbN", [D, NT])
        s_b1s = load("b1s", [WID, 3])
        s_b1N = load("b1N", [D, 1])
        s_G1 = load("G1", [WID, 32, WID], BF16)
        s_W2T = load("W2T", [WID, WID], BF16)
        s_w3 = load("w3pad", [WID, 32, 32], BF16)
        s_fw0 = load("fw0", [D, WID])
        s_fw1 = load("fw1", [WID, WID])
        s_fw2 = load("fw2", [WID, WID])
        s_fw3 = load("fw3", [WID, D])
        s_klc = load("klc", [WID, 1])
        s_prc = load("prc", [D, 1])

        logp = pslog.tile([1, S], F32)

        # ---- gate sets ----
        def emit_gates(j):
            """gate tiles for t_j: g0..g2, c0..c2 [WID,S]; g3 compact,
            g3 padded, c3 [D,S]."""
            gs, cs = [], []
            for i in range(3):
                g = gp.tile([WID, S], F32, tag=f"g{i}")
                nc.scalar.activation(g, s_ub[:, i, :], AF.Sigmoid,
                                     bias=s_gb[:, j * 3 + i:j * 3 + i + 1])
                c = gp.tile([WID, S], F32, tag=f"c{i}")
                nc.scalar.activation(c, s_cb[:, i, :], AF.Identity,
                                     bias=s_cbias[:, j * 3 + i:j * 3 + i + 1])
                gs.append(g)
                cs.append(c)
            g3 = gp.tile([D, S], F32, tag="g3")
            nc.scalar.activation(g3, s_ubN, AF.Sigmoid,
                                 bias=s_gbN[:, j:j + 1])
            g3pad = g3p.tile([WID, S], F32, tag="g3pad")
            nc.scalar.activation(g3pad, s_ubNp, AF.Sigmoid,
                                 bias=s_gbNp[:, j:j + 1])
            c3 = gp.tile([D, S], F32, tag="c3")
            nc.scalar.activation(c3, s_cbN, AF.Identity,
                                 bias=s_cbN2[:, j:j + 1])
            return dict(gs=gs, cs=cs, g3=g3, g3pad=g3pad, c3=c3)

        # ---- forward pass of the vector field ----
        def emit_fwd(psi_in, gset, drive):
            """returns (kp [D,S] f32, us bf16). `drive(n)` advances pending
            chain emission between the serial stages so PE/DVE/ACT stay fed."""
            gs, cs = gset["gs"], gset["cs"]
            us = []
            h = psi_in
            fws = [s_fw0, s_fw1, s_fw2]
            for i in range(3):
                pre = ps1.tile([WID, S], F32, tag="ch1")
                nc.tensor.matmul(pre, fws[i], h, start=True, stop=True)
                drive(4)
                z = fwp.tile([WID, S], F32, tag=f"z{i}")
                nc.vector.scalar_tensor_tensor(
                    z, pre, s_b1s[:, i:i + 1], gs[i], OP.add, OP.mult)
                zc = fwp.tile([WID, S], F32, tag=f"zc{i}")
                nc.gpsimd.tensor_tensor(zc, z, cs[i], OP.add)
                h = fwp.tile([WID, S], F32, tag=f"h{i}")
                nc.scalar.activation(h, zc, AF.Tanh)
                drive(4)
                hsq = fwp.tile([WID, S], F32, tag=f"hsq{i}")
                nc.gpsimd.tensor_tensor(hsq, h, h, OP.mult)
                u = up.tile([WID, S], BF16, tag=f"u{i}")
                nc.vector.scalar_tensor_tensor(
                    u, hsq, 1.0, gs[i], OP.subtract, OP.mult)
                us.append(u)
            pre3 = ps2.tile([D, S], F32, tag="ch2")
            nc.tensor.matmul(pre3, s_fw3, h, start=True, stop=True)
            drive(2)
            dp = fwp.tile([D, S], F32, tag="dp")
            nc.vector.scalar_tensor_tensor(
                dp, pre3, s_b1N, gset["g3"], OP.add, OP.mult)
            kp = fwp.tile([D, S], F32, tag="kp")
            nc.gpsimd.tensor_tensor(kp, dp, gset["c3"], OP.add)
            return kp, us

        # ---- eviction router: PSUM q -> SBUF bf16 (v = u * q) ----
        # 'd': direct DVE TT (1x)   'q': ACT copy + DVE bf16 2x TT
        # 'p': ACT copy + Pool TT
        def emit_evict(route, qps, u, shape, vtag, ctag):
            v = vp.tile(shape, BF16, tag=vtag)
            if len(shape) == 3:
                u = u.unsqueeze(1).broadcast_to(shape)
            if route == "d":
                nc.vector.tensor_tensor(v, qps, u, OP.mult)
            else:
                tmp = vp.tile(shape, BF16, tag=ctag)
                nc.scalar.copy(tmp, qps)
                if route == "q":
                    nc.vector.tensor_tensor(v, tmp, u, OP.mult)
                else:
                    nc.gpsimd.tensor_tensor(v, tmp, u, OP.mult)
            return v

        # ---- trace chain for one rhs eval (generator; yields per seed) ----
        def gen_chain(us, gset, first):
            u0p, u1p, u2p = us
            ch = CFG["ch1"]
            tau = pstau.tile([WID, S], F32, tag="tau")
            # stage 1 processes seed groups of `ch`; seed a -> col group
            # (a%4)*32, lhsT col a//4; order cycles the 4 groups
            for c in range(32 // ch):
                q1 = ps1.tile([WID, ch, S], F32, tag="ch1")
                for j in range(ch):
                    nc.tensor.matmul(q1[:, j, :], s_G1[:, ch * c + j, :], u0p,
                                     start=True, stop=True)
                v1 = emit_evict(ROUTES1[c % len(ROUTES1)], q1, u1p,
                                [WID, ch, S], "v1", "c1")
                for j in range(ch):
                    a = ch * c + j
                    q2 = ps2.tile([WID, S], F32, tag="ch2")
                    nc.tensor.matmul(q2, s_W2T, v1[:, j, :],
                                     start=True, stop=True)
                    v2 = emit_evict(ROUTES2[a], q2, u2p, [WID, S], "v2", "c2")
                    grp = (a % 4) * 32
                    nc.tensor.matmul(tau[grp:grp + 32, :], s_w3[:, a, :], v2,
                                     start=(a // 4 == 0), stop=(a // 4 == 7),
                                     tile_position=(0, grp))
                    yield
            r = rp.tile([WID, S], F32, tag="r")
            nc.vector.tensor_tensor(r, tau, gset["g3pad"], OP.mult)
            nc.tensor.matmul(logp, s_klc, r, start=first, stop=False)

        # ---- integration loop (software-pipelined emission) ----
        outsb = sing.tile([1, S], F32)
        for rep in range(reps):
            gsets = {0: emit_gates(0)}
            psiT = s_xT
            gens = []  # chain generators pending interleaved emission

            def drive(n):
                for _ in range(n):
                    if not gens:
                        return
                    if next(gens[0], "end") == "end":
                        gens.pop(0)

            for k in range(NSTEPS):
                k1p, us1 = emit_fwd(psiT, gsets[k], drive)
                gsets[k + 1] = emit_gates(k + 1)
                drive(6)
                pmid = psip.tile([D, S], F32, tag="pmid")
                nc.vector.scalar_tensor_tensor(
                    pmid, k1p, float(H), psiT, OP.mult, OP.add)
                k2p, us2 = emit_fwd(pmid, gsets[k + 1], drive)
                ksum = psip.tile([D, S], F32, tag="ksum")
                nc.gpsimd.tensor_tensor(ksum, k1p, k2p, OP.add)
                pnew = psip.tile([D, S], F32, tag="psiT")
                nc.vector.scalar_tensor_tensor(
                    pnew, ksum, float(0.5 * H), psiT, OP.mult, OP.add)
                psiT = pnew
                gens.append(gen_chain(us1, gsets[k], k == 0))
                gens.append(gen_chain(us2, gsets[k + 1], False))
                del gsets[k]
                # keep at most ~2 chains pending; drain the excess now
                while len(gens) > 2:
                    if next(gens[0], "end") == "end":
                        gens.pop(0)
            while gens:
                if next(gens[0], "end") == "end":
                    gens.pop(0)

            # ---- prior + output ----
            zsq = fwp.tile([D, S], F32, tag="zsq")
            nc.scalar.activation(zsq, psiT, AF.Square)
            nc.tensor.matmul(logp, s_prc, zsq, start=False, stop=True)
            nc.vector.tensor_scalar_add(outsb, logp, float(-0.5 * D * LOG2PI))
        nc.sync.dma_start(out=io["out_d"][:], in_=outsb)


def _prepare_inputs(inputs):
    """Host-side precompute. Returns per-core in_maps list."""
    f = lambda k: np.asarray(inputs[k], np.float32)
    x, cond = f("x"), f("cond")
    W0, W1m, W2m, W3 = f("l0_W1"), f("mid_W1")[0], f("mid_W1")[1], f("lN_W1")
    W2g = [f("l0_W2"), f("mid_W2")[0], f("mid_W2")[1], f("lN_W2")]
    b2g = [f("l0_b2"), f("mid_b2")[0], f("mid_b2")[1], f("lN_b2")]
    W3c = [f("l0_W3"), f("mid_W3")[0], f("mid_W3")[1], f("lN_W3")]
    b1 = [f("l0_b1"), f("mid_b1")[0], f("mid_b1")[1], f("lN_b1")]

    bf = ml_dtypes.bfloat16
    ts = (T1 + H * np.arange(NT)).astype(np.float32)        # [NT]

    # shared tensors
    shared = {}
    shared["G1"] = np.einsum("cd,da->dac", W1m, W0).astype(bf).copy()
    shared["W2T"] = W2m.T.astype(bf).copy()
    w3pad = np.zeros((WID, 32, 32), np.float32)
    for a in range(32):
        w3pad[:, a, a // 4] = W3[a, :]
    shared["w3pad"] = w3pad.astype(bf)
    shared["fw0"] = W0.T.copy()
    shared["fw1"] = W1m.T.copy()
    shared["fw2"] = W2m.T.copy()
    shared["fw3"] = W3.T.copy()
    shared["b1s"] = np.stack([b1[0], b1[1], b1[2]], axis=1).copy()
    shared["b1N"] = b1[3][:, None].copy()
    # per-t biases: gate bias = t*W2[:,0] + b2 ; c bias = t*W3[:,0]
    gbl = np.stack([np.outer(W2g[i][:, 0], ts) + b2g[i][:, None]
                    for i in range(3)], axis=2)             # [WID, NT, 3]
    shared["gb"] = gbl.reshape(WID, NT * 3).copy()
    cbl = np.stack([np.outer(W3c[i][:, 0], ts)
                    for i in range(3)], axis=2)
    shared["cb"] = cbl.reshape(WID, NT * 3).copy()
    shared["gbN"] = (np.outer(W2g[3][:, 0], ts) + b2g[3][:, None]).copy()
    shared["cbN"] = np.outer(W3c[3][:, 0], ts).copy()
    # tau row for seed a: col group (a%4)*32 + lhsT col (a//4)
    rowmap = np.array([32 * (a % 4) + a // 4 for a in range(32)])
    assert len(set(rowmap.tolist())) == 32
    gbNp = np.zeros((WID, NT), np.float32)
    klc = np.zeros((WID, 1), np.float32)
    ubNp = np.zeros((WID, B), np.float32)
    for a in range(32):
        gbNp[rowmap[a]] = shared["gbN"][a]
        klc[rowmap[a], 0] = 0.5 * DT   # +0.025  (absorbs chain sign flip)
    shared["gbNp"] = gbNp
    shared["klc"] = klc
    shared["prc"] = np.full((D, 1), -0.5, np.float32)

    # per-core tensors
    condT = cond.T                                          # [8, B]
    ub_all = np.stack([W2g[i][:, 1:] @ condT for i in range(3)], axis=1)
    cb_all = np.stack([W3c[i][:, 1:] @ condT for i in range(3)], axis=1)
    ubN_all = W2g[3][:, 1:] @ condT                         # [D, B]
    cbN_all = W3c[3][:, 1:] @ condT
    for a in range(32):
        ubNp[rowmap[a]] = ubN_all[a]

    in_maps = []
    for c in range(NCORES):
        sl = slice(c * S, (c + 1) * S)
        m = dict(shared)
        m["xT"] = x[sl].T.copy()
        m["ubase"] = ub_all[:, :, sl].copy()
        m["ubaseN"] = ubN_all[:, sl].copy()
        m["ubaseNp"] = ubNp[:, sl].copy()
        m["cbase"] = cb_all[:, :, sl].copy()
        m["cbaseN"] = cbN_all[:, sl].copy()
        in_maps.append(m)
    return in_maps


def kernel(**inputs):
    if "nc" not in _compiled:
        _compiled["nc"] = _build_nc()
    nc = _compiled["nc"]
    in_maps = _prepare_inputs(inputs)
    res = run_bass_kernel_spmd(nc, in_maps, list(range(NCORES)))
    out = np.concatenate([res.results[c]["out"][0] for c in range(NCORES)])
    return out.astype(np.float32)


if __name__ == "__main__":
    import os
    if os.path.exists("/tmp/inputs_full.npz"):
        inp = dict(np.load("/tmp/inputs_full.npz"))
    else:
        import reference as ref
        inp = {k: np.asarray(v) for k, v in ref.setup_inputs().items()}
    got = kernel(**inp)
    print("kernel output", got[:4], got.shape)
